# revision 26
# baseline (speedup 1.0000x reference)
"""Multi-head causal attention on 8 Trainium2 NeuronCores.

Sharding: core c -> (batch b = c//2, head-half hh = c%2).  Each core computes
q/k/v projections for its 8 heads (column-sharded wq/wk/wv), causal attention,
and a full-width partial output projection (row-sharded wo).  Host sums the
two partials per batch and adds the bias.

Device-side layout trick: scores are computed transposed (scoresT[j, i]) so
that the softmax-weighted sum over keys (ctx) is a plain matmul with v as the
stationary operand.  Ones-columns baked alongside v produce the softmax
denominator replicated across 64 partitions in the same PSUM tile as ctx.
"""

import numpy as np

import concourse.bass as bass
import concourse.mybir as mybir
import concourse.tile as tile
from concourse import bacc
from concourse.bass_utils import run_bass_kernel_spmd

# Problem shape (hardcoded; kernel.py must be self-contained).
B, S, D, H = 4, 2048, 1024, 16
HD = D // H           # 64 head dim
NCORES = 8
EH = D // 2           # 512: per-core e-width (8 heads)
NHL = H // 2          # 8 local heads per core
SB = 512              # s-block (free dim of most matmuls)
NSB = S // SB         # 4
NST = S // 128        # 16 s-tiles / j-tiles
NEG = EH // 128       # 4 e-groups of 128 partitions
NKG = D // 128        # 8 d-groups (contraction tiles)
VROW = 4 * 192        # v_ext row: 4x [v_even(64) | ones(64) | v_odd(64)] = 768

F32 = mybir.dt.float32
F32R = mybir.dt.float32r
BF16 = mybir.dt.bfloat16
MMDT = F32R          # dtype for matmul inputs (BF16 or F32R)
import ml_dtypes
MMNP = ml_dtypes.bfloat16 if MMDT == BF16 else np.float32

TRACE = False
LAST_RESULT = None


def _build():
    nc = bacc.Bacc()

    xT_d = nc.dram_tensor("xt", [D, S], MMDT, kind="ExternalInput")
    wqT_d = nc.dram_tensor("wqt", [D, EH], MMDT, kind="ExternalInput")
    wkT_d = nc.dram_tensor("wkt", [D, EH], MMDT, kind="ExternalInput")
    wvT_d = nc.dram_tensor("wvt", [D, EH], MMDT, kind="ExternalInput")
    woT_d = nc.dram_tensor("wot", [EH, D], MMDT, kind="ExternalInput")
    masks_d = nc.dram_tensor("masks", [128, 128], MMDT, kind="ExternalInput")
    out_d = nc.dram_tensor("out", [S, D], F32, kind="ExternalOutput")
    scr_d = nc.dram_tensor("dscr", [4, NSB, 2, SB], F32)

    with tile.TileContext(nc) as tc:
        with (
            tc.tile_pool(name="persist", bufs=1) as persist,
            tc.tile_pool(name="accp", bufs=4, space="PSUM") as accp,
        ):
            qT = persist.tile([128, NEG, S], MMDT)      # [e-part, e-group, s]
            kT = persist.tile([128, NEG, S], MMDT)
            v_ext = persist.tile([128, NST, VROW], MMDT)  # [s-part, s-tile, row]

            # shared ones block between each (even, odd) head pair
            for st in range(NST):
                for p in range(4):
                    ones_ap = v_ext[:, st, p * 192 + 64 : p * 192 + 128]
                    if MMDT == F32R:
                        ones_ap = ones_ap.bitcast(F32)
                    nc.vector.memset(ones_ap, 1.0)

            # ---------------- Phase 1: projections ----------------
            with (
                tc.tile_pool(name="p1w", bufs=1) as p1w,
                tc.tile_pool(name="p1x", bufs=2) as p1x,
            ):
                w_q = p1w.tile([128, NKG, EH], MMDT)
                w_k = p1w.tile([128, NKG, EH], MMDT)
                w_v = p1w.tile([128, NKG, EH], MMDT)
                for kg in range(NKG):
                    sl = slice(kg * 128, (kg + 1) * 128)
                    nc.gpsimd.dma_start(out=w_q[:, kg, :], in_=wqT_d[sl, :])
                for kg in range(NKG):
                    sl = slice(kg * 128, (kg + 1) * 128)
                    nc.gpsimd.dma_start(out=w_k[:, kg, :], in_=wkT_d[sl, :])
                    nc.gpsimd.dma_start(out=w_v[:, kg, :], in_=wvT_d[sl, :])

                for sb in range(NSB):
                    ssl = slice(sb * SB, (sb + 1) * SB)
                    xts = p1x.tile([128, NKG, SB], MMDT, tag="xts")
                    for kg in range(NKG):
                        nc.sync.dma_start(
                            out=xts[:, kg, :],
                            in_=xT_d[kg * 128 : (kg + 1) * 128, ssl],
                        )
                    # qT / kT blocks: out [e-part(128 of group mt), s(512)]
                    for w_sb, dst in ((w_q, qT), (w_k, kT)):
                        for mt in range(NEG):
                            ps = accp.tile([128, SB], F32, tag="acc")
                            msl = slice(mt * 128, (mt + 1) * 128)
                            for kg in range(NKG):
                                nc.tensor.matmul(
                                    out=ps,
                                    lhsT=(w_sb[:, kg, msl]),
                                    rhs=(xts[:, kg, :]),
                                    start=(kg == 0),
                                    stop=(kg == NKG - 1),
                                )
                            nc.vector.tensor_copy(dst[:, mt, ssl], ps)
                    # v blocks: out [s-part(128 of tile st), e(512)]
                    for st4 in range(SB // 128):
                        st = sb * (SB // 128) + st4
                        ps = accp.tile([128, EH], F32, tag="acc")
                        xsl = slice(st4 * 128, (st4 + 1) * 128)
                        for kg in range(NKG):
                            nc.tensor.matmul(
                                out=ps,
                                lhsT=(xts[:, kg, xsl]),
                                rhs=(w_v[:, kg, :]),
                                start=(kg == 0),
                                stop=(kg == NKG - 1),
                            )
                        # psum cols: head h at [h*64, h*64+64); dest pair p:
                        # even head -> p*192, odd head -> p*192+128
                        psr = ps[:].rearrange("p (a c) -> p a c", c=128)
                        vst = v_ext[:, st, :].rearrange("p (a w) -> p a w", w=192)
                        nc.vector.tensor_copy(vst[:, :, 128:192], psr[:, :, 0:64])
                        nc.vector.tensor_copy(vst[:, :, 0:64], psr[:, :, 64:128])

            # ---------------- Phase 2 + 3: attention and output proj ----------------
            # i-blocks outer, head pairs inner; once an i-block has all 8
            # heads' context, its output-projection tiles run immediately so
            # phase-3 matmuls and output DMAs overlap the attention phase.
            with (
                tc.tile_pool(name="p2c", bufs=1) as p2c,
                tc.tile_pool(name="ctxp", bufs=1) as ctxp,
                tc.tile_pool(name="expp", bufs=4) as expp,
                tc.tile_pool(name="sp", bufs=2, space="PSUM") as sp,
                tc.tile_pool(name="smallp", bufs=2) as smallp,
                tc.tile_pool(name="p3", bufs=2) as p3,
            ):
                masks_sb = p2c.tile([128, 128], MMDT)
                nc.gpsimd.dma_start(out=masks_sb, in_=masks_d[:, :])
                woT_sb = p2c.tile([128, NEG, D], MMDT)
                for gg in range(NEG):
                    nc.gpsimd.dma_start(
                        out=woT_sb[:, gg, :],
                        in_=woT_d[gg * 128 : (gg + 1) * 128, :],
                    )

                ctxT = ctxp.tile([128, NEG, S], MMDT)
                ib_order = [0, 1, 2, 3]
                for ib_i, ib in enumerate(ib_order):
                    isl = slice(ib * SB, (ib + 1) * SB)
                    njt = 4 * (ib + 1)
                    for pr in range(4):
                        ps_c0 = accp.tile([128, SB], F32, tag="acc")
                        ps_c1 = accp.tile([128, SB], F32, tag="acc")

                        def scores(jt):
                            r = jt - 4 * ib
                            f0 = 128 * r if r > 0 else 0
                            jsl = slice(jt * 128, (jt + 1) * 128)
                            qsl = slice(ib * SB + f0, (ib + 1) * SB)
                            ps_s = sp.tile([128, 2 * SB], F32, tag="s")
                            nc.tensor.matmul(
                                out=ps_s[:, f0:SB],
                                lhsT=kT[0:64, pr, jsl],
                                rhs=qT[0:64, pr, qsl],
                                start=True,
                                stop=True,
                            )
                            nc.tensor.matmul(
                                out=ps_s[:, SB + f0 : 2 * SB],
                                lhsT=kT[64:128, pr, jsl],
                                rhs=qT[64:128, pr, qsl],
                                start=True,
                                stop=True,
                            )
                            return ps_s

                        def softmax_ctx(jt, ps_s):
                            r = jt - 4 * ib
                            f0 = 128 * r if r > 0 else 0
                            expT = expp.tile([128, 2 * SB], MMDT, tag="exp")
                            ps_v = ps_s[:].rearrange("p (t c) -> p t c", t=2)
                            ex_v = expT[:].rearrange("p (t c) -> p t c", t=2)
                            nc.scalar.activation(
                                out=ex_v[:, :, f0:SB],
                                in_=ps_v[:, :, f0:SB],
                                func=mybir.ActivationFunctionType.Exp,
                                scale=1.0 / np.sqrt(HD),
                            )
                            if r >= 0:
                                nc.gpsimd.tensor_mul(
                                    ex_v[:, :, f0 : f0 + 128],
                                    ex_v[:, :, f0 : f0 + 128],
                                    masks_sb[:].unsqueeze(1).broadcast_to(
                                        (128, 2, 128)
                                    ),
                                )
                            for t, ps_c in ((0, ps_c0), (1, ps_c1)):
                                coff = pr * 192 + (64 if t == 0 else 0)
                                nc.tensor.matmul(
                                    out=ps_c[:, f0:SB],
                                    lhsT=v_ext[:, jt, coff : coff + 128],
                                    rhs=expT[:, t * SB + f0 : (t + 1) * SB],
                                    start=(jt == 0),
                                    stop=(jt == njt - 1),
                                )

                        prev = None
                        for jt in range(njt):
                            ps_prev = prev
                            prev = (jt, scores(jt))
                            if ps_prev is not None:
                                softmax_ctx(*ps_prev)
                        softmax_ctx(*prev)

                        # even head (ps_c0): denom rows 0:64, ctx rows 64:128
                        den0 = smallp.tile([128, SB], F32, tag="den0")
                        nc.vector.tensor_copy(den0[0:64, :], ps_c0[0:64, :])
                        rdt0 = smallp.tile([128, SB], F32, tag="rdt0")
                        nc.vector.reciprocal_approx_fast(
                            rdt0[0:64, :], den0[0:64, :]
                        )
                        nc.sync.dma_start(
                            out=scr_d[pr, ib, 0, :], in_=rdt0[0:1, :]
                        )
                        se = scr_d[pr, ib, 0, :]
                        bce = smallp.tile([128, SB], F32, tag="bce")
                        nc.sync.dma_start(
                            out=bce[64:128, :],
                            in_=bass.AP(
                                tensor=se.tensor, offset=se.offset,
                                ap=[[0, 64], [1, SB]],
                            ),
                        )
                        nc.vector.tensor_mul(
                            ctxT[64:128, pr, isl], ps_c0[64:128, :], bce[64:128, :]
                        )
                        # odd head (ps_c1): ctx rows 0:64, denom rows 64:128
                        den1 = smallp.tile([128, SB], F32, tag="den1")
                        nc.vector.tensor_copy(den1[64:65, :], ps_c1[64:65, :])
                        nc.sync.dma_start(
                            out=scr_d[pr, ib, 1, :], in_=den1[64:65, :]
                        )
                        so = scr_d[pr, ib, 1, :]
                        braw = smallp.tile([128, SB], F32, tag="braw")
                        nc.sync.dma_start(
                            out=braw[0:64, :],
                            in_=bass.AP(
                                tensor=so.tensor, offset=so.offset,
                                ap=[[0, 64], [1, SB]],
                            ),
                        )
                        rdt1 = smallp.tile([128, SB], F32, tag="rdt1")
                        nc.vector.reciprocal_approx_fast(
                            rdt1[0:64, :], braw[0:64, :]
                        )
                        nc.vector.tensor_mul(
                            ctxT[0:64, pr, isl], ps_c1[0:64, :], rdt1[0:64, :]
                        )

                    # output projection, deferred one i-block so its
                    # dependencies (normalize chain) are already settled
                    ib_o = ib_order[ib_i - 1] if ib_i > 0 else None
                    for it in ([] if ib_o is None else range(4 * ib_o, 4 * ib_o + 4)):
                        itsl = slice(it * 128, (it + 1) * 128)
                        for ob in range(2):
                            osl = slice(ob * SB, (ob + 1) * SB)
                            ps = accp.tile([128, SB], F32, tag="acc")
                            for gg in range(NEG):
                                nc.tensor.matmul(
                                    out=ps,
                                    lhsT=(ctxT[:, gg, itsl]),
                                    rhs=(woT_sb[:, gg, osl]),
                                    start=(gg == 0),
                                    stop=(gg == NEG - 1),
                                )
                            ot = p3.tile([128, SB], F32, tag="ot")
                            nc.vector.tensor_copy(ot, ps)
                            nc.sync.dma_start(out=out_d[itsl, osl], in_=ot)

                # tail: output projection for the last-processed i-block
                for it in range(4 * ib_order[-1], 4 * ib_order[-1] + 4):
                    itsl = slice(it * 128, (it + 1) * 128)
                    for ob in range(2):
                        osl = slice(ob * SB, (ob + 1) * SB)
                        ps = accp.tile([128, SB], F32, tag="acc")
                        for gg in range(NEG):
                            nc.tensor.matmul(
                                out=ps,
                                lhsT=(ctxT[:, gg, itsl]),
                                rhs=(woT_sb[:, gg, osl]),
                                start=(gg == 0),
                                stop=(gg == NEG - 1),
                            )
                        ot = p3.tile([128, SB], F32, tag="ot")
                        nc.vector.tensor_copy(ot, ps)
                        nc.sync.dma_start(out=out_d[itsl, osl], in_=ot)

    nc.finalize()
    return nc


_NC = None


def _get_nc():
    global _NC
    if _NC is None:
        _NC = _build()
    return _NC


def kernel(x, wq, wk, wv, wo, wo_b):
    global LAST_RESULT
    x = np.ascontiguousarray(np.asarray(x, dtype=np.float32))
    wq = np.asarray(wq, dtype=np.float32)
    wk = np.asarray(wk, dtype=np.float32)
    wv = np.asarray(wv, dtype=np.float32)
    wo = np.asarray(wo, dtype=np.float32)
    wo_b = np.asarray(wo_b, dtype=np.float32)

    pp, ff = np.ogrid[0:128, 0:128]
    masks = (pp <= ff).astype(np.float32)

    in_maps = []
    for c in range(NCORES):
        b, hh = c // 2, c % 2
        es = slice(hh * EH, (hh + 1) * EH)
        in_maps.append(
            {
                "xt": np.ascontiguousarray(x[b].T.astype(MMNP)),
                "wqt": np.ascontiguousarray(wq[es, :].T.astype(MMNP)),
                "wkt": np.ascontiguousarray(wk[es, :].T.astype(MMNP)),
                "wvt": np.ascontiguousarray(wv[es, :].T.astype(MMNP)),
                "wot": np.ascontiguousarray(
                    wo[:, es].T.astype(MMNP)
                    .reshape(4, 2, 64, D)[:, ::-1]
                    .reshape(EH, D)
                ),
                "masks": masks.astype(MMNP),
            }
        )

    nc = _get_nc()
    res = run_bass_kernel_spmd(nc, in_maps, list(range(NCORES)), trace=TRACE)
    LAST_RESULT = res

    out = np.empty((B, S, D), np.float32)
    for b in range(B):
        out[b] = res.results[2 * b]["out"] + res.results[2 * b + 1]["out"]
    out += wo_b[None, None, :]
    return out


# revision 27
# speedup vs baseline: 1.1341x; 1.1341x over previous
"""Multi-head causal attention on 8 Trainium2 NeuronCores.

Sharding: core c -> (batch b = c//2, head-half hh = c%2).  Each core computes
q/k/v projections for its 8 heads (column-sharded wq/wk/wv), causal attention,
and a full-width partial output projection (row-sharded wo).  Host sums the
two partials per batch and adds the bias.

Device-side layout trick: scores are computed transposed (scoresT[j, i]) so
that the softmax-weighted sum over keys (ctx) is a plain matmul with v as the
stationary operand.  Ones-columns baked alongside v produce the softmax
denominator replicated across 64 partitions in the same PSUM tile as ctx.
"""

import numpy as np

import concourse.bass as bass
import concourse.mybir as mybir
import concourse.tile as tile
from concourse import bacc
from concourse.bass_utils import run_bass_kernel_spmd

# Problem shape (hardcoded; kernel.py must be self-contained).
B, S, D, H = 4, 2048, 1024, 16
HD = D // H           # 64 head dim
NCORES = 8
EH = D // 2           # 512: per-core e-width (8 heads)
NHL = H // 2          # 8 local heads per core
SB = 512              # s-block (free dim of most matmuls)
NSB = S // SB         # 4
NST = S // 128        # 16 s-tiles / j-tiles
NEG = EH // 128       # 4 e-groups of 128 partitions
NKG = D // 128        # 8 d-groups (contraction tiles)
VROW = 4 * 192        # v_ext row: 4x [v_even(64) | ones(64) | v_odd(64)] = 768

F32 = mybir.dt.float32
F32R = mybir.dt.float32r
BF16 = mybir.dt.bfloat16
MMDT = F32R          # dtype for matmul inputs (BF16 or F32R)
import ml_dtypes
MMNP = ml_dtypes.bfloat16 if MMDT == BF16 else np.float32

TRACE = False
LAST_RESULT = None


def _build():
    nc = bacc.Bacc()

    xT_d = nc.dram_tensor("xt", [D, S], MMDT, kind="ExternalInput")
    wqT_d = nc.dram_tensor("wqt", [D, EH], MMDT, kind="ExternalInput")
    wkT_d = nc.dram_tensor("wkt", [D, EH], MMDT, kind="ExternalInput")
    wvT_d = nc.dram_tensor("wvt", [D, EH], MMDT, kind="ExternalInput")
    woT_d = nc.dram_tensor("wot", [EH, D], MMDT, kind="ExternalInput")
    masks_d = nc.dram_tensor("masks", [128, 128], MMDT, kind="ExternalInput")
    out_d = nc.dram_tensor("out", [S, D], F32, kind="ExternalOutput")
    scr_d = nc.dram_tensor("dscr", [4, NSB, 2, SB], F32)

    with tile.TileContext(nc) as tc:
        with (
            tc.tile_pool(name="persist", bufs=1) as persist,
            tc.tile_pool(name="accp", bufs=4, space="PSUM") as accp,
        ):
            qT = persist.tile([128, NEG, S], MMDT)      # [e-part, e-group, s]
            kT = persist.tile([128, NEG, S], MMDT)
            v_ext = persist.tile([128, NST, VROW], MMDT)  # [s-part, s-tile, row]

            # shared ones block between each (even, odd) head pair
            for st in range(NST):
                for p in range(4):
                    ones_ap = v_ext[:, st, p * 192 + 64 : p * 192 + 128]
                    if MMDT == F32R:
                        ones_ap = ones_ap.bitcast(F32)
                    nc.vector.memset(ones_ap, 1.0)

            # ---------------- Phase 1: projections ----------------
            with (
                tc.tile_pool(name="p1w", bufs=1) as p1w,
                tc.tile_pool(name="p1x", bufs=2) as p1x,
            ):
                w_q = p1w.tile([128, NKG, EH], MMDT)
                w_k = p1w.tile([128, NKG, EH], MMDT)
                w_v = p1w.tile([128, NKG, EH], MMDT)
                for kg in range(NKG):
                    sl = slice(kg * 128, (kg + 1) * 128)
                    nc.gpsimd.dma_start(out=w_q[:, kg, :], in_=wqT_d[sl, :])
                for kg in range(NKG):
                    sl = slice(kg * 128, (kg + 1) * 128)
                    nc.gpsimd.dma_start(out=w_k[:, kg, :], in_=wkT_d[sl, :])
                    nc.gpsimd.dma_start(out=w_v[:, kg, :], in_=wvT_d[sl, :])

                for sb in range(NSB):
                    ssl = slice(sb * SB, (sb + 1) * SB)
                    xts = p1x.tile([128, NKG, SB], MMDT, tag="xts")
                    for kg in range(NKG):
                        nc.sync.dma_start(
                            out=xts[:, kg, :],
                            in_=xT_d[kg * 128 : (kg + 1) * 128, ssl],
                        )
                    # qT / kT blocks: out [e-part(128 of group mt), s(512)]
                    for w_sb, dst in ((w_q, qT), (w_k, kT)):
                        for mt in range(NEG):
                            ps = accp.tile([128, SB], F32, tag="acc")
                            msl = slice(mt * 128, (mt + 1) * 128)
                            for kg in range(NKG):
                                nc.tensor.matmul(
                                    out=ps,
                                    lhsT=(w_sb[:, kg, msl]),
                                    rhs=(xts[:, kg, :]),
                                    start=(kg == 0),
                                    stop=(kg == NKG - 1),
                                )
                            nc.vector.tensor_copy(dst[:, mt, ssl], ps)
                    # v blocks: out [s-part(128 of tile st), e(512)]
                    for st4 in range(SB // 128):
                        st = sb * (SB // 128) + st4
                        ps = accp.tile([128, EH], F32, tag="acc")
                        xsl = slice(st4 * 128, (st4 + 1) * 128)
                        for kg in range(NKG):
                            nc.tensor.matmul(
                                out=ps,
                                lhsT=(xts[:, kg, xsl]),
                                rhs=(w_v[:, kg, :]),
                                start=(kg == 0),
                                stop=(kg == NKG - 1),
                            )
                        # psum cols: head h at [h*64, h*64+64); dest pair p:
                        # even head -> p*192, odd head -> p*192+128
                        psr = ps[:].rearrange("p (a c) -> p a c", c=128)
                        vst = v_ext[:, st, :].rearrange("p (a w) -> p a w", w=192)
                        nc.vector.tensor_copy(vst[:, :, 128:192], psr[:, :, 0:64])
                        nc.vector.tensor_copy(vst[:, :, 0:64], psr[:, :, 64:128])

            # ---------------- Phase 2 + 3: attention and output proj ----------------
            # i-blocks outer, head pairs inner; once an i-block has all 8
            # heads' context, its output-projection tiles run immediately so
            # phase-3 matmuls and output DMAs overlap the attention phase.
            with (
                tc.tile_pool(name="p2c", bufs=1) as p2c,
                tc.tile_pool(name="ctxp", bufs=1) as ctxp,
                tc.tile_pool(name="expp", bufs=4) as expp,
                tc.tile_pool(name="sp", bufs=2, space="PSUM") as sp,
                tc.tile_pool(name="smallp", bufs=2) as smallp,
                tc.tile_pool(name="p3", bufs=2) as p3,
            ):
                masks_sb = p2c.tile([128, 128], MMDT)
                nc.gpsimd.dma_start(out=masks_sb, in_=masks_d[:, :])
                woT_sb = p2c.tile([128, NEG, D], MMDT)
                for gg in range(NEG):
                    nc.gpsimd.dma_start(
                        out=woT_sb[:, gg, :],
                        in_=woT_d[gg * 128 : (gg + 1) * 128, :],
                    )

                ctxT = ctxp.tile([128, NEG, S], MMDT)
                ib_order = [1, 2, 3, 0]
                for ib_i, ib in enumerate(ib_order):
                    isl = slice(ib * SB, (ib + 1) * SB)
                    njt = 4 * (ib + 1)
                    for pr in range(4):
                        ps_c0 = accp.tile([128, SB], F32, tag="acc")
                        ps_c1 = accp.tile([128, SB], F32, tag="acc")

                        def scores(jt):
                            r = jt - 4 * ib
                            f0 = 128 * r if r > 0 else 0
                            jsl = slice(jt * 128, (jt + 1) * 128)
                            qsl = slice(ib * SB + f0, (ib + 1) * SB)
                            ps_s = sp.tile([128, 2 * SB], F32, tag="s")
                            nc.tensor.matmul(
                                out=ps_s[:, f0:SB],
                                lhsT=kT[0:64, pr, jsl],
                                rhs=qT[0:64, pr, qsl],
                                start=True,
                                stop=True,
                            )
                            nc.tensor.matmul(
                                out=ps_s[:, SB + f0 : 2 * SB],
                                lhsT=kT[64:128, pr, jsl],
                                rhs=qT[64:128, pr, qsl],
                                start=True,
                                stop=True,
                            )
                            return ps_s

                        def softmax_ctx(jt, ps_s):
                            r = jt - 4 * ib
                            f0 = 128 * r if r > 0 else 0
                            expT = expp.tile([128, 2 * SB], MMDT, tag="exp")
                            ps_v = ps_s[:].rearrange("p (t c) -> p t c", t=2)
                            ex_v = expT[:].rearrange("p (t c) -> p t c", t=2)
                            nc.scalar.activation(
                                out=ex_v[:, :, f0:SB],
                                in_=ps_v[:, :, f0:SB],
                                func=mybir.ActivationFunctionType.Exp,
                                scale=1.0 / np.sqrt(HD),
                            )
                            if r >= 0:
                                nc.vector.tensor_mul(
                                    ex_v[:, :, f0 : f0 + 128],
                                    ex_v[:, :, f0 : f0 + 128],
                                    masks_sb[:].unsqueeze(1).broadcast_to(
                                        (128, 2, 128)
                                    ),
                                )
                            for t, ps_c in ((0, ps_c0), (1, ps_c1)):
                                coff = pr * 192 + (64 if t == 0 else 0)
                                nc.tensor.matmul(
                                    out=ps_c[:, f0:SB],
                                    lhsT=v_ext[:, jt, coff : coff + 128],
                                    rhs=expT[:, t * SB + f0 : (t + 1) * SB],
                                    start=(jt == 0),
                                    stop=(jt == njt - 1),
                                )

                        prev = None
                        for jt in range(njt):
                            ps_prev = prev
                            prev = (jt, scores(jt))
                            if ps_prev is not None:
                                softmax_ctx(*ps_prev)
                        softmax_ctx(*prev)

                        # even head (ps_c0): denom rows 0:64, ctx rows 64:128
                        den0 = smallp.tile([128, SB], F32, tag="den0")
                        nc.vector.tensor_copy(den0[0:64, :], ps_c0[0:64, :])
                        rdt0 = smallp.tile([128, SB], F32, tag="rdt0")
                        nc.vector.reciprocal_approx_fast(
                            rdt0[0:64, :], den0[0:64, :]
                        )
                        nc.sync.dma_start(
                            out=scr_d[pr, ib, 0, :], in_=rdt0[0:1, :]
                        )
                        se = scr_d[pr, ib, 0, :]
                        bce = smallp.tile([128, SB], F32, tag="bce")
                        nc.sync.dma_start(
                            out=bce[64:128, :],
                            in_=bass.AP(
                                tensor=se.tensor, offset=se.offset,
                                ap=[[0, 64], [1, SB]],
                            ),
                        )
                        nc.vector.tensor_mul(
                            ctxT[64:128, pr, isl], ps_c0[64:128, :], bce[64:128, :]
                        )
                        # odd head (ps_c1): ctx rows 0:64, denom rows 64:128
                        den1 = smallp.tile([128, SB], F32, tag="den1")
                        nc.vector.tensor_copy(den1[64:65, :], ps_c1[64:65, :])
                        nc.sync.dma_start(
                            out=scr_d[pr, ib, 1, :], in_=den1[64:65, :]
                        )
                        so = scr_d[pr, ib, 1, :]
                        braw = smallp.tile([128, SB], F32, tag="braw")
                        nc.sync.dma_start(
                            out=braw[0:64, :],
                            in_=bass.AP(
                                tensor=so.tensor, offset=so.offset,
                                ap=[[0, 64], [1, SB]],
                            ),
                        )
                        rdt1 = smallp.tile([128, SB], F32, tag="rdt1")
                        nc.vector.reciprocal_approx_fast(
                            rdt1[0:64, :], braw[0:64, :]
                        )
                        nc.vector.tensor_mul(
                            ctxT[0:64, pr, isl], ps_c1[0:64, :], rdt1[0:64, :]
                        )

                    # output projection, deferred one i-block so its
                    # dependencies (normalize chain) are already settled
                    ib_o = ib_order[ib_i - 1] if ib_i > 0 else None
                    for it in ([] if ib_o is None else range(4 * ib_o, 4 * ib_o + 4)):
                        itsl = slice(it * 128, (it + 1) * 128)
                        for ob in range(2):
                            osl = slice(ob * SB, (ob + 1) * SB)
                            ps = accp.tile([128, SB], F32, tag="acc")
                            for gg in range(NEG):
                                nc.tensor.matmul(
                                    out=ps,
                                    lhsT=(ctxT[:, gg, itsl]),
                                    rhs=(woT_sb[:, gg, osl]),
                                    start=(gg == 0),
                                    stop=(gg == NEG - 1),
                                )
                            ot = p3.tile([128, SB], F32, tag="ot")
                            nc.vector.tensor_copy(ot, ps)
                            nc.sync.dma_start(out=out_d[itsl, osl], in_=ot)

                # tail: output projection for the last-processed i-block
                for it in range(4 * ib_order[-1], 4 * ib_order[-1] + 4):
                    itsl = slice(it * 128, (it + 1) * 128)
                    for ob in range(2):
                        osl = slice(ob * SB, (ob + 1) * SB)
                        ps = accp.tile([128, SB], F32, tag="acc")
                        for gg in range(NEG):
                            nc.tensor.matmul(
                                out=ps,
                                lhsT=(ctxT[:, gg, itsl]),
                                rhs=(woT_sb[:, gg, osl]),
                                start=(gg == 0),
                                stop=(gg == NEG - 1),
                            )
                        ot = p3.tile([128, SB], F32, tag="ot")
                        nc.vector.tensor_copy(ot, ps)
                        nc.sync.dma_start(out=out_d[itsl, osl], in_=ot)

    nc.finalize()
    return nc


_NC = None


def _get_nc():
    global _NC
    if _NC is None:
        _NC = _build()
    return _NC


def kernel(x, wq, wk, wv, wo, wo_b):
    global LAST_RESULT
    x = np.ascontiguousarray(np.asarray(x, dtype=np.float32))
    wq = np.asarray(wq, dtype=np.float32)
    wk = np.asarray(wk, dtype=np.float32)
    wv = np.asarray(wv, dtype=np.float32)
    wo = np.asarray(wo, dtype=np.float32)
    wo_b = np.asarray(wo_b, dtype=np.float32)

    pp, ff = np.ogrid[0:128, 0:128]
    masks = (pp <= ff).astype(np.float32)

    in_maps = []
    for c in range(NCORES):
        b, hh = c // 2, c % 2
        es = slice(hh * EH, (hh + 1) * EH)
        in_maps.append(
            {
                "xt": np.ascontiguousarray(x[b].T.astype(MMNP)),
                "wqt": np.ascontiguousarray(wq[es, :].T.astype(MMNP)),
                "wkt": np.ascontiguousarray(wk[es, :].T.astype(MMNP)),
                "wvt": np.ascontiguousarray(wv[es, :].T.astype(MMNP)),
                "wot": np.ascontiguousarray(
                    wo[:, es].T.astype(MMNP)
                    .reshape(4, 2, 64, D)[:, ::-1]
                    .reshape(EH, D)
                ),
                "masks": masks.astype(MMNP),
            }
        )

    nc = _get_nc()
    res = run_bass_kernel_spmd(nc, in_maps, list(range(NCORES)), trace=TRACE)
    LAST_RESULT = res

    out = np.empty((B, S, D), np.float32)
    for b in range(B):
        out[b] = res.results[2 * b]["out"] + res.results[2 * b + 1]["out"]
    out += wo_b[None, None, :]
    return out


# revision 28
# speedup vs baseline: 1.1604x; 1.0232x over previous
"""Multi-head causal attention on 8 Trainium2 NeuronCores.

Sharding: core c -> (batch b = c//2, head-half hh = c%2).  Each core computes
q/k/v projections for its 8 heads (column-sharded wq/wk/wv), causal attention,
and a full-width partial output projection (row-sharded wo).  Host sums the
two partials per batch and adds the bias.

Device-side layout trick: scores are computed transposed (scoresT[j, i]) so
that the softmax-weighted sum over keys (ctx) is a plain matmul with v as the
stationary operand.  Ones-columns baked alongside v produce the softmax
denominator replicated across 64 partitions in the same PSUM tile as ctx.
"""

import numpy as np

import concourse.bass as bass
import concourse.mybir as mybir
import concourse.tile as tile
from concourse import bacc
from concourse.bass_utils import run_bass_kernel_spmd

# Problem shape (hardcoded; kernel.py must be self-contained).
B, S, D, H = 4, 2048, 1024, 16
HD = D // H           # 64 head dim
NCORES = 8
EH = D // 2           # 512: per-core e-width (8 heads)
NHL = H // 2          # 8 local heads per core
SB = 512              # s-block (free dim of most matmuls)
NSB = S // SB         # 4
NST = S // 128        # 16 s-tiles / j-tiles
NEG = EH // 128       # 4 e-groups of 128 partitions
NKG = D // 128        # 8 d-groups (contraction tiles)
VROW = 4 * 192        # v_ext row: 4x [v_even(64) | ones(64) | v_odd(64)] = 768

F32 = mybir.dt.float32
F32R = mybir.dt.float32r
BF16 = mybir.dt.bfloat16
MMDT = F32R          # dtype for matmul inputs (BF16 or F32R)
import ml_dtypes
MMNP = ml_dtypes.bfloat16 if MMDT == BF16 else np.float32

TRACE = False
LAST_RESULT = None


def _build():
    nc = bacc.Bacc()

    xT_d = nc.dram_tensor("xt", [D, S], MMDT, kind="ExternalInput")
    wqT_d = nc.dram_tensor("wqt", [D, EH], MMDT, kind="ExternalInput")
    wkT_d = nc.dram_tensor("wkt", [D, EH], MMDT, kind="ExternalInput")
    wvT_d = nc.dram_tensor("wvt", [D, EH], MMDT, kind="ExternalInput")
    woT_d = nc.dram_tensor("wot", [EH, D], MMDT, kind="ExternalInput")
    masks_d = nc.dram_tensor("masks", [128, 128], MMDT, kind="ExternalInput")
    out_d = nc.dram_tensor("out", [S, D], F32, kind="ExternalOutput")
    scr_d = nc.dram_tensor("dscr", [4, NSB, 2, SB], F32)

    with tile.TileContext(nc) as tc:
        with (
            tc.tile_pool(name="persist", bufs=1) as persist,
            tc.tile_pool(name="accp", bufs=4, space="PSUM") as accp,
        ):
            qT = persist.tile([128, NEG, S], MMDT)      # [e-part, e-group, s]
            kT = persist.tile([128, NEG, S], MMDT)
            v_ext = persist.tile([128, NST, VROW], MMDT)  # [s-part, s-tile, row]

            # shared ones block between each (even, odd) head pair
            for st in range(NST):
                for p in range(4):
                    ones_ap = v_ext[:, st, p * 192 + 64 : p * 192 + 128]
                    if MMDT == F32R:
                        ones_ap = ones_ap.bitcast(F32)
                    nc.vector.memset(ones_ap, 1.0)

            # ---------------- Phase 1: projections ----------------
            with (
                tc.tile_pool(name="p1w", bufs=1) as p1w,
                tc.tile_pool(name="p1x", bufs=2) as p1x,
            ):
                w_q = p1w.tile([128, NKG, EH], MMDT)
                w_k = p1w.tile([128, NKG, EH], MMDT)
                w_v = p1w.tile([128, NKG, EH], MMDT)
                for kg in range(NKG):
                    sl = slice(kg * 128, (kg + 1) * 128)
                    nc.gpsimd.dma_start(out=w_q[:, kg, :], in_=wqT_d[sl, :])
                for kg in range(NKG):
                    sl = slice(kg * 128, (kg + 1) * 128)
                    nc.gpsimd.dma_start(out=w_k[:, kg, :], in_=wkT_d[sl, :])
                    nc.gpsimd.dma_start(out=w_v[:, kg, :], in_=wvT_d[sl, :])

                for sb in range(NSB):
                    ssl = slice(sb * SB, (sb + 1) * SB)
                    xts = p1x.tile([128, NKG, SB], MMDT, tag="xts")
                    for kg in range(NKG):
                        nc.sync.dma_start(
                            out=xts[:, kg, :],
                            in_=xT_d[kg * 128 : (kg + 1) * 128, ssl],
                        )
                    # qT / kT blocks: out [e-part(128 of group mt), s(512)]
                    for w_sb, dst in ((w_q, qT), (w_k, kT)):
                        for mt in range(NEG):
                            ps = accp.tile([128, SB], F32, tag="acc")
                            msl = slice(mt * 128, (mt + 1) * 128)
                            for kg in range(NKG):
                                nc.tensor.matmul(
                                    out=ps,
                                    lhsT=(w_sb[:, kg, msl]),
                                    rhs=(xts[:, kg, :]),
                                    start=(kg == 0),
                                    stop=(kg == NKG - 1),
                                )
                            nc.vector.tensor_copy(dst[:, mt, ssl], ps)
                    # v blocks: out [s-part(128 of tile st), e(512)]
                    for st4 in range(SB // 128):
                        st = sb * (SB // 128) + st4
                        ps = accp.tile([128, EH], F32, tag="acc")
                        xsl = slice(st4 * 128, (st4 + 1) * 128)
                        for kg in range(NKG):
                            nc.tensor.matmul(
                                out=ps,
                                lhsT=(xts[:, kg, xsl]),
                                rhs=(w_v[:, kg, :]),
                                start=(kg == 0),
                                stop=(kg == NKG - 1),
                            )
                        # psum cols: head h at [h*64, h*64+64); dest pair p:
                        # even head -> p*192, odd head -> p*192+128
                        psr = ps[:].rearrange("p (a c) -> p a c", c=128)
                        vst = v_ext[:, st, :].rearrange("p (a w) -> p a w", w=192)
                        nc.vector.tensor_copy(vst[:, :, 128:192], psr[:, :, 0:64])
                        nc.vector.tensor_copy(vst[:, :, 0:64], psr[:, :, 64:128])

            # ---------------- Phase 2 + 3: attention and output proj ----------------
            # i-blocks outer, head pairs inner; once an i-block has all 8
            # heads' context, its output-projection tiles run immediately so
            # phase-3 matmuls and output DMAs overlap the attention phase.
            with (
                tc.tile_pool(name="p2c", bufs=1) as p2c,
                tc.tile_pool(name="ctxp", bufs=1) as ctxp,
                tc.tile_pool(name="expp", bufs=4) as expp,
                tc.tile_pool(name="sp", bufs=2, space="PSUM") as sp,
                tc.tile_pool(name="smallp", bufs=2) as smallp,
                tc.tile_pool(name="p3", bufs=2) as p3,
            ):
                masks_sb = p2c.tile([128, 128], MMDT)
                nc.gpsimd.dma_start(out=masks_sb, in_=masks_d[:, :])
                woT_sb = p2c.tile([128, NEG, D], MMDT)
                for gg in range(NEG):
                    nc.gpsimd.dma_start(
                        out=woT_sb[:, gg, :],
                        in_=woT_d[gg * 128 : (gg + 1) * 128, :],
                    )

                ctxT = ctxp.tile([128, NEG, S], MMDT)
                ib_order = [0, 1, 2, 3]
                for ib_i, ib in enumerate(ib_order):
                    isl = slice(ib * SB, (ib + 1) * SB)
                    njt = 4 * (ib + 1)
                    for pr in range(4):
                        ps_c0 = accp.tile([128, SB], F32, tag="acc")
                        ps_c1 = accp.tile([128, SB], F32, tag="acc")

                        def scores(jt):
                            r = jt - 4 * ib
                            f0 = 128 * r if r > 0 else 0
                            jsl = slice(jt * 128, (jt + 1) * 128)
                            qsl = slice(ib * SB + f0, (ib + 1) * SB)
                            ps_s = sp.tile([128, 2 * SB], F32, tag="s")
                            nc.tensor.matmul(
                                out=ps_s[:, f0:SB],
                                lhsT=kT[0:64, pr, jsl],
                                rhs=qT[0:64, pr, qsl],
                                start=True,
                                stop=True,
                            )
                            nc.tensor.matmul(
                                out=ps_s[:, SB + f0 : 2 * SB],
                                lhsT=kT[64:128, pr, jsl],
                                rhs=qT[64:128, pr, qsl],
                                start=True,
                                stop=True,
                            )
                            return ps_s

                        def softmax_ctx(jt, ps_s):
                            r = jt - 4 * ib
                            f0 = 128 * r if r > 0 else 0
                            expT = expp.tile([128, 2 * SB], MMDT, tag="exp")
                            ps_v = ps_s[:].rearrange("p (t c) -> p t c", t=2)
                            ex_v = expT[:].rearrange("p (t c) -> p t c", t=2)
                            nc.scalar.activation(
                                out=ex_v[:, :, f0:SB],
                                in_=ps_v[:, :, f0:SB],
                                func=mybir.ActivationFunctionType.Exp,
                                scale=1.0 / np.sqrt(HD),
                            )
                            if r >= 0:
                                nc.vector.tensor_mul(
                                    ex_v[:, :, f0 : f0 + 128],
                                    ex_v[:, :, f0 : f0 + 128],
                                    masks_sb[:].unsqueeze(1).broadcast_to(
                                        (128, 2, 128)
                                    ),
                                )
                            for t, ps_c in ((0, ps_c0), (1, ps_c1)):
                                coff = pr * 192 + (64 if t == 0 else 0)
                                nc.tensor.matmul(
                                    out=ps_c[:, f0:SB],
                                    lhsT=v_ext[:, jt, coff : coff + 128],
                                    rhs=expT[:, t * SB + f0 : (t + 1) * SB],
                                    start=(jt == 0),
                                    stop=(jt == njt - 1),
                                )

                        prev = None
                        for jt in range(njt):
                            ps_prev = prev
                            prev = (jt, scores(jt))
                            if ps_prev is not None:
                                softmax_ctx(*ps_prev)
                        softmax_ctx(*prev)

                        # even head (ps_c0): denom rows 0:64, ctx rows 64:128
                        den0 = smallp.tile([128, SB], F32, tag="den0")
                        nc.vector.tensor_copy(den0[0:64, :], ps_c0[0:64, :])
                        rdt0 = smallp.tile([128, SB], F32, tag="rdt0")
                        nc.vector.reciprocal_approx_fast(
                            rdt0[0:64, :], den0[0:64, :]
                        )
                        nc.sync.dma_start(
                            out=scr_d[pr, ib, 0, :], in_=rdt0[0:1, :]
                        )
                        se = scr_d[pr, ib, 0, :]
                        bce = smallp.tile([128, SB], F32, tag="bce")
                        nc.sync.dma_start(
                            out=bce[64:128, :],
                            in_=bass.AP(
                                tensor=se.tensor, offset=se.offset,
                                ap=[[0, 64], [1, SB]],
                            ),
                        )
                        nc.vector.tensor_mul(
                            ctxT[64:128, pr, isl], ps_c0[64:128, :], bce[64:128, :]
                        )
                        # odd head (ps_c1): ctx rows 0:64, denom rows 64:128
                        den1 = smallp.tile([128, SB], F32, tag="den1")
                        nc.vector.tensor_copy(den1[64:65, :], ps_c1[64:65, :])
                        nc.sync.dma_start(
                            out=scr_d[pr, ib, 1, :], in_=den1[64:65, :]
                        )
                        so = scr_d[pr, ib, 1, :]
                        braw = smallp.tile([128, SB], F32, tag="braw")
                        nc.sync.dma_start(
                            out=braw[0:64, :],
                            in_=bass.AP(
                                tensor=so.tensor, offset=so.offset,
                                ap=[[0, 64], [1, SB]],
                            ),
                        )
                        rdt1 = smallp.tile([128, SB], F32, tag="rdt1")
                        nc.vector.reciprocal_approx_fast(
                            rdt1[0:64, :], braw[0:64, :]
                        )
                        nc.vector.tensor_mul(
                            ctxT[0:64, pr, isl], ps_c1[0:64, :], rdt1[0:64, :]
                        )

                    # output projection, deferred one i-block so its
                    # dependencies (normalize chain) are already settled
                    ib_o = ib_order[ib_i - 1] if ib_i > 0 else None
                    for it in ([] if ib_o is None else range(4 * ib_o, 4 * ib_o + 4)):
                        itsl = slice(it * 128, (it + 1) * 128)
                        for ob in range(2):
                            osl = slice(ob * SB, (ob + 1) * SB)
                            ps = accp.tile([128, SB], F32, tag="acc")
                            for gg in range(NEG):
                                nc.tensor.matmul(
                                    out=ps,
                                    lhsT=(ctxT[:, gg, itsl]),
                                    rhs=(woT_sb[:, gg, osl]),
                                    start=(gg == 0),
                                    stop=(gg == NEG - 1),
                                )
                            ot = p3.tile([128, SB], F32, tag="ot")
                            nc.vector.tensor_copy(ot, ps)
                            nc.sync.dma_start(out=out_d[itsl, osl], in_=ot)

                # tail: output projection for the last-processed i-block
                for it in range(4 * ib_order[-1], 4 * ib_order[-1] + 4):
                    itsl = slice(it * 128, (it + 1) * 128)
                    for ob in range(2):
                        osl = slice(ob * SB, (ob + 1) * SB)
                        ps = accp.tile([128, SB], F32, tag="acc")
                        for gg in range(NEG):
                            nc.tensor.matmul(
                                out=ps,
                                lhsT=(ctxT[:, gg, itsl]),
                                rhs=(woT_sb[:, gg, osl]),
                                start=(gg == 0),
                                stop=(gg == NEG - 1),
                            )
                        ot = p3.tile([128, SB], F32, tag="ot")
                        nc.vector.tensor_copy(ot, ps)
                        nc.sync.dma_start(out=out_d[itsl, osl], in_=ot)

    nc.finalize()
    return nc


_NC = None


def _get_nc():
    global _NC
    if _NC is None:
        _NC = _build()
    return _NC


def kernel(x, wq, wk, wv, wo, wo_b):
    global LAST_RESULT
    x = np.ascontiguousarray(np.asarray(x, dtype=np.float32))
    wq = np.asarray(wq, dtype=np.float32)
    wk = np.asarray(wk, dtype=np.float32)
    wv = np.asarray(wv, dtype=np.float32)
    wo = np.asarray(wo, dtype=np.float32)
    wo_b = np.asarray(wo_b, dtype=np.float32)

    pp, ff = np.ogrid[0:128, 0:128]
    masks = (pp <= ff).astype(np.float32)

    in_maps = []
    for c in range(NCORES):
        b, hh = c // 2, c % 2
        es = slice(hh * EH, (hh + 1) * EH)
        in_maps.append(
            {
                "xt": np.ascontiguousarray(x[b].T.astype(MMNP)),
                "wqt": np.ascontiguousarray(wq[es, :].T.astype(MMNP)),
                "wkt": np.ascontiguousarray(wk[es, :].T.astype(MMNP)),
                "wvt": np.ascontiguousarray(wv[es, :].T.astype(MMNP)),
                "wot": np.ascontiguousarray(
                    wo[:, es].T.astype(MMNP)
                    .reshape(4, 2, 64, D)[:, ::-1]
                    .reshape(EH, D)
                ),
                "masks": masks.astype(MMNP),
            }
        )

    nc = _get_nc()
    res = run_bass_kernel_spmd(nc, in_maps, list(range(NCORES)), trace=TRACE)
    LAST_RESULT = res

    out = np.empty((B, S, D), np.float32)
    for b in range(B):
        out[b] = res.results[2 * b]["out"] + res.results[2 * b + 1]["out"]
    out += wo_b[None, None, :]
    return out


# revision 30
# speedup vs baseline: 1.1630x; 1.0022x over previous
"""Multi-head causal attention on 8 Trainium2 NeuronCores.

Sharding: core c -> (batch b = c//2, head-half hh = c%2).  Each core computes
q/k/v projections for its 8 heads (column-sharded wq/wk/wv), causal attention,
and a full-width partial output projection (row-sharded wo).  Host sums the
two partials per batch and adds the bias.

Device-side layout trick: scores are computed transposed (scoresT[j, i]) so
that the softmax-weighted sum over keys (ctx) is a plain matmul with v as the
stationary operand.  Ones-columns baked alongside v produce the softmax
denominator replicated across 64 partitions in the same PSUM tile as ctx.
"""

import numpy as np

import concourse.bass as bass
import concourse.mybir as mybir
import concourse.tile as tile
from concourse import bacc
from concourse.bass_utils import run_bass_kernel_spmd

# Problem shape (hardcoded; kernel.py must be self-contained).
B, S, D, H = 4, 2048, 1024, 16
HD = D // H           # 64 head dim
NCORES = 8
EH = D // 2           # 512: per-core e-width (8 heads)
NHL = H // 2          # 8 local heads per core
SB = 512              # s-block (free dim of most matmuls)
NSB = S // SB         # 4
NST = S // 128        # 16 s-tiles / j-tiles
NEG = EH // 128       # 4 e-groups of 128 partitions
NKG = D // 128        # 8 d-groups (contraction tiles)
VROW = 4 * 192        # v_ext row: 4x [v_even(64) | ones(64) | v_odd(64)] = 768

F32 = mybir.dt.float32
F32R = mybir.dt.float32r
BF16 = mybir.dt.bfloat16
MMDT = F32R          # dtype for matmul inputs (BF16 or F32R)
import ml_dtypes
MMNP = ml_dtypes.bfloat16 if MMDT == BF16 else np.float32

TRACE = False
LAST_RESULT = None


def _build():
    nc = bacc.Bacc()

    xT_d = nc.dram_tensor("xt", [D, S], MMDT, kind="ExternalInput")
    wqT_d = nc.dram_tensor("wqt", [D, EH], MMDT, kind="ExternalInput")
    wkT_d = nc.dram_tensor("wkt", [D, EH], MMDT, kind="ExternalInput")
    wvT_d = nc.dram_tensor("wvt", [D, EH], MMDT, kind="ExternalInput")
    woT_d = nc.dram_tensor("wot", [EH, D], MMDT, kind="ExternalInput")
    masks_d = nc.dram_tensor("masks", [128, 128], MMDT, kind="ExternalInput")
    out_d = nc.dram_tensor("out", [S, D], F32, kind="ExternalOutput")
    scr_d = nc.dram_tensor("dscr", [4, NSB, 2, SB], F32)

    with tile.TileContext(nc) as tc:
        with (
            tc.tile_pool(name="persist", bufs=1) as persist,
            tc.tile_pool(name="accp", bufs=4, space="PSUM") as accp,
        ):
            qT = persist.tile([128, NEG, S], MMDT)      # [e-part, e-group, s]
            kT = persist.tile([128, NEG, S], MMDT)
            v_ext = persist.tile([128, NST, VROW], MMDT)  # [s-part, s-tile, row]

            # shared ones block between each (even, odd) head pair
            for st in range(NST):
                for p in range(4):
                    ones_ap = v_ext[:, st, p * 192 + 64 : p * 192 + 128]
                    if MMDT == F32R:
                        ones_ap = ones_ap.bitcast(F32)
                    nc.vector.memset(ones_ap, 1.0)

            # ---------------- Phase 1: projections ----------------
            with (
                tc.tile_pool(name="p1w", bufs=1) as p1w,
                tc.tile_pool(name="p1x", bufs=2) as p1x,
            ):
                w_q = p1w.tile([128, NKG, EH], MMDT)
                w_k = p1w.tile([128, NKG, EH], MMDT)
                w_v = p1w.tile([128, NKG, EH], MMDT)
                for kg in range(NKG):
                    sl = slice(kg * 128, (kg + 1) * 128)
                    nc.gpsimd.dma_start(out=w_q[:, kg, :], in_=wqT_d[sl, :])
                for kg in range(NKG):
                    sl = slice(kg * 128, (kg + 1) * 128)
                    nc.gpsimd.dma_start(out=w_k[:, kg, :], in_=wkT_d[sl, :])
                    nc.gpsimd.dma_start(out=w_v[:, kg, :], in_=wvT_d[sl, :])

                for sb in range(NSB):
                    ssl = slice(sb * SB, (sb + 1) * SB)
                    xts = p1x.tile([128, NKG, SB], MMDT, tag="xts")
                    for kg in range(NKG):
                        nc.sync.dma_start(
                            out=xts[:, kg, :],
                            in_=xT_d[kg * 128 : (kg + 1) * 128, ssl],
                        )
                    # qT / kT blocks: out [e-part(128 of group mt), s(512)]
                    for w_sb, dst in ((w_q, qT), (w_k, kT)):
                        for mt in range(NEG):
                            ps = accp.tile([128, SB], F32, tag="acc")
                            msl = slice(mt * 128, (mt + 1) * 128)
                            for kg in range(NKG):
                                nc.tensor.matmul(
                                    out=ps,
                                    lhsT=(w_sb[:, kg, msl]),
                                    rhs=(xts[:, kg, :]),
                                    start=(kg == 0),
                                    stop=(kg == NKG - 1),
                                )
                            nc.vector.tensor_copy(dst[:, mt, ssl], ps)
                    # v blocks: out [s-part(128 of tile st), e(512)]
                    for st4 in range(SB // 128):
                        st = sb * (SB // 128) + st4
                        ps = accp.tile([128, EH], F32, tag="acc")
                        xsl = slice(st4 * 128, (st4 + 1) * 128)
                        for kg in range(NKG):
                            nc.tensor.matmul(
                                out=ps,
                                lhsT=(xts[:, kg, xsl]),
                                rhs=(w_v[:, kg, :]),
                                start=(kg == 0),
                                stop=(kg == NKG - 1),
                            )
                        # psum cols: head h at [h*64, h*64+64); dest pair p:
                        # even head -> p*192, odd head -> p*192+128
                        psr = ps[:].rearrange("p (a c) -> p a c", c=128)
                        vst = v_ext[:, st, :].rearrange("p (a w) -> p a w", w=192)
                        nc.vector.tensor_copy(vst[:, :, 128:192], psr[:, :, 0:64])
                        nc.vector.tensor_copy(vst[:, :, 0:64], psr[:, :, 64:128])

            # ---------------- Phase 2 + 3: attention and output proj ----------------
            # i-blocks outer, head pairs inner; once an i-block has all 8
            # heads' context, its output-projection tiles run immediately so
            # phase-3 matmuls and output DMAs overlap the attention phase.
            with (
                tc.tile_pool(name="p2c", bufs=1) as p2c,
                tc.tile_pool(name="ctxp", bufs=1) as ctxp,
                tc.tile_pool(name="expp", bufs=4) as expp,
                tc.tile_pool(name="sp", bufs=2, space="PSUM") as sp,
                tc.tile_pool(name="smallp", bufs=2) as smallp,
                tc.tile_pool(name="p3", bufs=2) as p3,
            ):
                masks_sb = p2c.tile([128, 128], MMDT)
                nc.gpsimd.dma_start(out=masks_sb, in_=masks_d[:, :])
                woT_sb = p2c.tile([128, NEG, D], MMDT)
                for gg in range(NEG):
                    nc.gpsimd.dma_start(
                        out=woT_sb[:, gg, :],
                        in_=woT_d[gg * 128 : (gg + 1) * 128, :],
                    )

                ctxT = ctxp.tile([128, NEG, S], MMDT)
                ib_order = [0, 1, 2, 3]
                for ib_i, ib in enumerate(ib_order):
                    isl = slice(ib * SB, (ib + 1) * SB)
                    njt = 4 * (ib + 1)
                    for pr in range(4):
                        ps_c0 = accp.tile([128, SB], F32, tag="acc")
                        ps_c1 = accp.tile([128, SB], F32, tag="acc")

                        def scores(jt):
                            r = jt - 4 * ib
                            f0 = 128 * r if r > 0 else 0
                            jsl = slice(jt * 128, (jt + 1) * 128)
                            qsl = slice(ib * SB + f0, (ib + 1) * SB)
                            ps_s = sp.tile([128, 2 * SB], F32, tag="s")
                            nc.tensor.matmul(
                                out=ps_s[:, f0:SB],
                                lhsT=kT[0:64, pr, jsl],
                                rhs=qT[0:64, pr, qsl],
                                start=True,
                                stop=True,
                            )
                            nc.tensor.matmul(
                                out=ps_s[:, SB + f0 : 2 * SB],
                                lhsT=kT[64:128, pr, jsl],
                                rhs=qT[64:128, pr, qsl],
                                start=True,
                                stop=True,
                            )
                            return ps_s

                        def softmax_ctx(jt, ps_s):
                            r = jt - 4 * ib
                            f0 = 128 * r if r > 0 else 0
                            expT = expp.tile([128, 2 * SB], MMDT, tag="exp")
                            ps_v = ps_s[:].rearrange("p (t c) -> p t c", t=2)
                            ex_v = expT[:].rearrange("p (t c) -> p t c", t=2)
                            nc.scalar.activation(
                                out=ex_v[:, :, f0:SB],
                                in_=ps_v[:, :, f0:SB],
                                func=mybir.ActivationFunctionType.Exp,
                                scale=1.0 / np.sqrt(HD),
                            )
                            if r >= 0:
                                nc.vector.tensor_mul(
                                    ex_v[:, :, f0 : f0 + 128],
                                    ex_v[:, :, f0 : f0 + 128],
                                    masks_sb[:].unsqueeze(1).broadcast_to(
                                        (128, 2, 128)
                                    ),
                                )
                            for t, ps_c in ((0, ps_c0), (1, ps_c1)):
                                coff = pr * 192 + (64 if t == 0 else 0)
                                nc.tensor.matmul(
                                    out=ps_c[:, f0:SB],
                                    lhsT=v_ext[:, jt, coff : coff + 128],
                                    rhs=expT[:, t * SB + f0 : (t + 1) * SB],
                                    start=(jt == 0),
                                    stop=(jt == njt - 1),
                                )

                        prev = None
                        for jt in range(njt):
                            ps_prev = prev
                            prev = (jt, scores(jt))
                            if ps_prev is not None:
                                softmax_ctx(*ps_prev)
                        softmax_ctx(*prev)

                        # even head (ps_c0): denom rows 0:64, ctx rows 64:128
                        den0 = smallp.tile([128, SB], F32, tag="den0")
                        nc.vector.tensor_copy(den0[0:64, :], ps_c0[0:64, :])
                        rdt0 = smallp.tile([128, SB], F32, tag="rdt0")
                        nc.vector.reciprocal_approx_fast(
                            rdt0[0:64, :], den0[0:64, :]
                        )
                        nc.sync.dma_start(
                            out=scr_d[pr, ib, 0, :], in_=rdt0[0:1, :]
                        )
                        se = scr_d[pr, ib, 0, :]
                        bce = smallp.tile([128, SB], F32, tag="bce")
                        nc.sync.dma_start(
                            out=bce[64:128, :],
                            in_=bass.AP(
                                tensor=se.tensor, offset=se.offset,
                                ap=[[0, 64], [1, SB]],
                            ),
                        )
                        nc.vector.tensor_mul(
                            ctxT[64:128, pr, isl], ps_c0[64:128, :], bce[64:128, :]
                        )
                        # odd head (ps_c1): ctx rows 0:64, denom rows 64:128
                        den1 = smallp.tile([128, SB], F32, tag="den1")
                        nc.vector.tensor_copy(den1[64:65, :], ps_c1[64:65, :])
                        nc.sync.dma_start(
                            out=scr_d[pr, ib, 1, :], in_=den1[64:65, :]
                        )
                        so = scr_d[pr, ib, 1, :]
                        braw = smallp.tile([128, SB], F32, tag="braw")
                        nc.sync.dma_start(
                            out=braw[0:64, :],
                            in_=bass.AP(
                                tensor=so.tensor, offset=so.offset,
                                ap=[[0, 64], [1, SB]],
                            ),
                        )
                        rdt1 = smallp.tile([128, SB], F32, tag="rdt1")
                        nc.vector.reciprocal_approx_fast(
                            rdt1[0:64, :], braw[0:64, :]
                        )
                        nc.vector.tensor_mul(
                            ctxT[0:64, pr, isl], ps_c1[0:64, :], rdt1[0:64, :]
                        )

                    # output projection, deferred one i-block so its
                    # dependencies (normalize chain) are already settled
                    ib_o = ib_order[ib_i - 1] if ib_i > 0 else None
                    for it in ([] if ib_o is None else range(4 * ib_o, 4 * ib_o + 4)):
                        itsl = slice(it * 128, (it + 1) * 128)
                        for ob in range(2):
                            osl = slice(ob * SB, (ob + 1) * SB)
                            ps = accp.tile([128, SB], F32, tag="acc")
                            for gg in range(NEG):
                                nc.tensor.matmul(
                                    out=ps,
                                    lhsT=(ctxT[:, gg, itsl]),
                                    rhs=(woT_sb[:, gg, osl]),
                                    start=(gg == 0),
                                    stop=(gg == NEG - 1),
                                )
                            ot = p3.tile([128, SB], F32, tag="ot")
                            nc.vector.tensor_copy(ot, ps)
                            nc.sync.dma_start(out=out_d[itsl, osl], in_=ot)

                # tail: output projection for the last-processed i-block
                for it in range(4 * ib_order[-1], 4 * ib_order[-1] + 4):
                    itsl = slice(it * 128, (it + 1) * 128)
                    for ob in range(2):
                        osl = slice(ob * SB, (ob + 1) * SB)
                        ps = accp.tile([128, SB], F32, tag="acc")
                        for gg in range(NEG):
                            nc.tensor.matmul(
                                out=ps,
                                lhsT=(ctxT[:, gg, itsl]),
                                rhs=(woT_sb[:, gg, osl]),
                                start=(gg == 0),
                                stop=(gg == NEG - 1),
                            )
                        ot = p3.tile([128, SB], F32, tag="ot")
                        nc.vector.tensor_copy(ot, ps)
                        nc.sync.dma_start(out=out_d[itsl, osl], in_=ot)

    nc.finalize()
    return nc


_NC = None


def _get_nc():
    global _NC
    if _NC is None:
        _NC = _build()
    return _NC


def kernel(x, wq, wk, wv, wo, wo_b):
    global LAST_RESULT
    x = np.ascontiguousarray(np.asarray(x, dtype=np.float32))
    wq = np.asarray(wq, dtype=np.float32)
    wk = np.asarray(wk, dtype=np.float32)
    wv = np.asarray(wv, dtype=np.float32)
    wo = np.asarray(wo, dtype=np.float32)
    wo_b = np.asarray(wo_b, dtype=np.float32)

    pp, ff = np.ogrid[0:128, 0:128]
    masks = (pp <= ff).astype(np.float32)

    in_maps = []
    for c in range(NCORES):
        b, hh = c // 2, c % 2
        es = slice(hh * EH, (hh + 1) * EH)
        in_maps.append(
            {
                "xt": np.ascontiguousarray(x[b].T.astype(MMNP)),
                "wqt": np.ascontiguousarray(wq[es, :].T.astype(MMNP)),
                "wkt": np.ascontiguousarray(wk[es, :].T.astype(MMNP)),
                "wvt": np.ascontiguousarray(wv[es, :].T.astype(MMNP)),
                "wot": np.ascontiguousarray(
                    wo[:, es].T.astype(MMNP)
                    .reshape(4, 2, 64, D)[:, ::-1]
                    .reshape(EH, D)
                ),
                "masks": masks.astype(MMNP),
            }
        )

    nc = _get_nc()
    res = run_bass_kernel_spmd(nc, in_maps, list(range(NCORES)), trace=TRACE)
    LAST_RESULT = res

    out = np.empty((B, S, D), np.float32)
    for b in range(B):
        out[b] = res.results[2 * b]["out"] + res.results[2 * b + 1]["out"]
    out += wo_b[None, None, :]
    return out


# revision 31
# speedup vs baseline: 1.1742x; 1.0096x over previous
"""Multi-head causal attention on 8 Trainium2 NeuronCores.

Sharding: core c -> (batch b = c//2, head-half hh = c%2).  Each core computes
q/k/v projections for its 8 heads (column-sharded wq/wk/wv), causal attention,
and a full-width partial output projection (row-sharded wo).  Host sums the
two partials per batch and adds the bias.

Device-side layout trick: scores are computed transposed (scoresT[j, i]) so
that the softmax-weighted sum over keys (ctx) is a plain matmul with v as the
stationary operand.  Ones-columns baked alongside v produce the softmax
denominator replicated across 64 partitions in the same PSUM tile as ctx.
"""

import numpy as np

import concourse.bass as bass
import concourse.mybir as mybir
import concourse.tile as tile
from concourse import bacc
from concourse.bass_utils import run_bass_kernel_spmd

# Problem shape (hardcoded; kernel.py must be self-contained).
B, S, D, H = 4, 2048, 1024, 16
HD = D // H           # 64 head dim
NCORES = 8
EH = D // 2           # 512: per-core e-width (8 heads)
NHL = H // 2          # 8 local heads per core
SB = 512              # s-block (free dim of most matmuls)
NSB = S // SB         # 4
NST = S // 128        # 16 s-tiles / j-tiles
NEG = EH // 128       # 4 e-groups of 128 partitions
NKG = D // 128        # 8 d-groups (contraction tiles)
VROW = 4 * 192        # v_ext row: 4x [v_even(64) | ones(64) | v_odd(64)] = 768

F32 = mybir.dt.float32
F32R = mybir.dt.float32r
BF16 = mybir.dt.bfloat16
MMDT = F32R          # dtype for matmul inputs (BF16 or F32R)
import ml_dtypes
MMNP = ml_dtypes.bfloat16 if MMDT == BF16 else np.float32

TRACE = False
LAST_RESULT = None


def _build():
    nc = bacc.Bacc()

    xT_d = nc.dram_tensor("xt", [D, S], MMDT, kind="ExternalInput")
    wqT_d = nc.dram_tensor("wqt", [D, EH], MMDT, kind="ExternalInput")
    wkT_d = nc.dram_tensor("wkt", [D, EH], MMDT, kind="ExternalInput")
    wvT_d = nc.dram_tensor("wvt", [D, EH], MMDT, kind="ExternalInput")
    woT_d = nc.dram_tensor("wot", [EH, D], MMDT, kind="ExternalInput")
    masks_d = nc.dram_tensor("masks", [128, 128], MMDT, kind="ExternalInput")
    out_d = nc.dram_tensor("out", [S, D], F32, kind="ExternalOutput")
    scr_d = nc.dram_tensor("dscr", [4, NSB, 2, SB], F32)

    with tile.TileContext(nc) as tc:
        with (
            tc.tile_pool(name="persist", bufs=1) as persist,
            tc.tile_pool(name="accp", bufs=4, space="PSUM") as accp,
        ):
            qT = persist.tile([128, NEG, S], MMDT)      # [e-part, e-group, s]
            kT = persist.tile([128, NEG, S], MMDT)
            v_ext = persist.tile([128, NST, VROW], MMDT)  # [s-part, s-tile, row]

            # shared ones block between each (even, odd) head pair
            for st in range(NST):
                for p in range(4):
                    ones_ap = v_ext[:, st, p * 192 + 64 : p * 192 + 128]
                    if MMDT == F32R:
                        ones_ap = ones_ap.bitcast(F32)
                    nc.vector.memset(ones_ap, 1.0)

            # ---------------- Phase 1: projections ----------------
            with (
                tc.tile_pool(name="p1w", bufs=1) as p1w,
                tc.tile_pool(name="p1x", bufs=2) as p1x,
            ):
                w_q = p1w.tile([128, NKG, EH], MMDT)
                w_k = p1w.tile([128, NKG, EH], MMDT)
                w_v = p1w.tile([128, NKG, EH], MMDT)
                for kg in range(NKG):
                    sl = slice(kg * 128, (kg + 1) * 128)
                    nc.gpsimd.dma_start(out=w_q[:, kg, :], in_=wqT_d[sl, :])
                for kg in range(NKG):
                    sl = slice(kg * 128, (kg + 1) * 128)
                    nc.gpsimd.dma_start(out=w_k[:, kg, :], in_=wkT_d[sl, :])
                    nc.gpsimd.dma_start(out=w_v[:, kg, :], in_=wvT_d[sl, :])

                for sb in range(NSB):
                    ssl = slice(sb * SB, (sb + 1) * SB)
                    xts = p1x.tile([128, NKG, SB], MMDT, tag="xts")
                    for kg in range(NKG):
                        nc.sync.dma_start(
                            out=xts[:, kg, :],
                            in_=xT_d[kg * 128 : (kg + 1) * 128, ssl],
                        )
                    # qT / kT blocks: out [e-part(128 of group mt), s(512)]
                    for w_sb, dst in ((w_q, qT), (w_k, kT)):
                        for mt in range(NEG):
                            ps = accp.tile([128, SB], F32, tag="acc")
                            msl = slice(mt * 128, (mt + 1) * 128)
                            for kg in range(NKG):
                                nc.tensor.matmul(
                                    out=ps,
                                    lhsT=(w_sb[:, kg, msl]),
                                    rhs=(xts[:, kg, :]),
                                    start=(kg == 0),
                                    stop=(kg == NKG - 1),
                                )
                            nc.vector.tensor_copy(dst[:, mt, ssl], ps)
                    # v blocks: out [s-part(128 of tile st), e(512)]
                    for st4 in range(SB // 128):
                        st = sb * (SB // 128) + st4
                        ps = accp.tile([128, EH], F32, tag="acc")
                        xsl = slice(st4 * 128, (st4 + 1) * 128)
                        for kg in range(NKG):
                            nc.tensor.matmul(
                                out=ps,
                                lhsT=(xts[:, kg, xsl]),
                                rhs=(w_v[:, kg, :]),
                                start=(kg == 0),
                                stop=(kg == NKG - 1),
                            )
                        # psum cols: head h at [h*64, h*64+64); dest pair p:
                        # even head -> p*192, odd head -> p*192+128
                        psr = ps[:].rearrange("p (a c) -> p a c", c=128)
                        vst = v_ext[:, st, :].rearrange("p (a w) -> p a w", w=192)
                        nc.vector.tensor_copy(vst[:, :, 128:192], psr[:, :, 0:64])
                        nc.vector.tensor_copy(vst[:, :, 0:64], psr[:, :, 64:128])

            # ---------------- Phase 2 + 3: attention and output proj ----------------
            # i-blocks outer, head pairs inner; once an i-block has all 8
            # heads' context, its output-projection tiles run immediately so
            # phase-3 matmuls and output DMAs overlap the attention phase.
            with (
                tc.tile_pool(name="p2c", bufs=1) as p2c,
                tc.tile_pool(name="ctxp", bufs=1) as ctxp,
                tc.tile_pool(name="expp", bufs=4) as expp,
                tc.tile_pool(name="sp", bufs=2, space="PSUM") as sp,
                tc.tile_pool(name="smallp", bufs=2) as smallp,
                tc.tile_pool(name="p3", bufs=2) as p3,
            ):
                masks_sb = p2c.tile([128, 128], MMDT)
                nc.gpsimd.dma_start(out=masks_sb, in_=masks_d[:, :])
                woT_sb = p2c.tile([128, NEG, D], MMDT)
                for gg in range(NEG):
                    nc.gpsimd.dma_start(
                        out=woT_sb[:, gg, :],
                        in_=woT_d[gg * 128 : (gg + 1) * 128, :],
                    )

                ctxT = ctxp.tile([128, NEG, S], MMDT)
                ib_order = [0, 1, 2, 3]
                for ib_i, ib in enumerate(ib_order):
                    isl = slice(ib * SB, (ib + 1) * SB)
                    njt = 4 * (ib + 1)
                    for pr in range(4):
                        ps_c0 = accp.tile([128, SB], F32, tag="acc")
                        ps_c1 = accp.tile([128, SB], F32, tag="acc")

                        def scores(jt):
                            r = jt - 4 * ib
                            f0 = 128 * r if r > 0 else 0
                            jsl = slice(jt * 128, (jt + 1) * 128)
                            qsl = slice(ib * SB + f0, (ib + 1) * SB)
                            ps_s = sp.tile([128, 2 * SB], F32, tag="s")
                            nc.tensor.matmul(
                                out=ps_s[:, f0:SB],
                                lhsT=kT[0:64, pr, jsl],
                                rhs=qT[0:64, pr, qsl],
                                start=True,
                                stop=True,
                            )
                            nc.tensor.matmul(
                                out=ps_s[:, SB + f0 : 2 * SB],
                                lhsT=kT[64:128, pr, jsl],
                                rhs=qT[64:128, pr, qsl],
                                start=True,
                                stop=True,
                            )
                            return ps_s

                        def softmax_ctx(jt, ps_s):
                            r = jt - 4 * ib
                            f0 = 128 * r if r > 0 else 0
                            expT = expp.tile([128, 2 * SB], MMDT, tag="exp")
                            ps_v = ps_s[:].rearrange("p (t c) -> p t c", t=2)
                            ex_v = expT[:].rearrange("p (t c) -> p t c", t=2)
                            nc.scalar.activation(
                                out=ex_v[:, :, f0:SB],
                                in_=ps_v[:, :, f0:SB],
                                func=mybir.ActivationFunctionType.Exp,
                                scale=1.0 / np.sqrt(HD),
                            )
                            if r >= 0:
                                nc.vector.tensor_mul(
                                    ex_v[:, :, f0 : f0 + 128],
                                    ex_v[:, :, f0 : f0 + 128],
                                    masks_sb[:].unsqueeze(1).broadcast_to(
                                        (128, 2, 128)
                                    ),
                                )
                            for t, ps_c in ((0, ps_c0), (1, ps_c1)):
                                coff = pr * 192 + (64 if t == 0 else 0)
                                nc.tensor.matmul(
                                    out=ps_c[:, f0:SB],
                                    lhsT=v_ext[:, jt, coff : coff + 128],
                                    rhs=expT[:, t * SB + f0 : (t + 1) * SB],
                                    start=(jt == 0),
                                    stop=(jt == njt - 1),
                                )

                        prev = None
                        for jt in range(njt):
                            ps_prev = prev
                            prev = (jt, scores(jt))
                            if ps_prev is not None:
                                softmax_ctx(*ps_prev)
                        softmax_ctx(*prev)

                        # even head (ps_c0): denom rows 0:64, ctx rows 64:128
                        den0 = smallp.tile([128, SB], F32, tag="den0")
                        nc.vector.tensor_copy(den0[0:64, :], ps_c0[0:64, :])
                        rdt0 = smallp.tile([128, SB], F32, tag="rdt0")
                        nc.vector.reciprocal_approx_fast(
                            rdt0[0:64, :], den0[0:64, :]
                        )
                        nc.sync.dma_start(
                            out=scr_d[pr, ib, 0, :], in_=rdt0[0:1, :]
                        )
                        se = scr_d[pr, ib, 0, :]
                        bce = smallp.tile([128, SB], F32, tag="bce")
                        nc.sync.dma_start(
                            out=bce[64:128, :],
                            in_=bass.AP(
                                tensor=se.tensor, offset=se.offset,
                                ap=[[0, 64], [1, SB]],
                            ),
                        )
                        nc.vector.tensor_mul(
                            ctxT[64:128, pr, isl], ps_c0[64:128, :], bce[64:128, :]
                        )
                        # odd head (ps_c1): ctx rows 0:64, denom rows 64:128
                        den1 = smallp.tile([128, SB], F32, tag="den1")
                        nc.vector.tensor_copy(den1[64:65, :], ps_c1[64:65, :])
                        nc.sync.dma_start(
                            out=scr_d[pr, ib, 1, :], in_=den1[64:65, :]
                        )
                        so = scr_d[pr, ib, 1, :]
                        braw = smallp.tile([128, SB], F32, tag="braw")
                        nc.sync.dma_start(
                            out=braw[0:64, :],
                            in_=bass.AP(
                                tensor=so.tensor, offset=so.offset,
                                ap=[[0, 64], [1, SB]],
                            ),
                        )
                        rdt1 = smallp.tile([128, SB], F32, tag="rdt1")
                        nc.vector.reciprocal_approx_fast(
                            rdt1[0:64, :], braw[0:64, :]
                        )
                        nc.vector.tensor_mul(
                            ctxT[0:64, pr, isl], ps_c1[0:64, :], rdt1[0:64, :]
                        )

                        # one deferred out-projection tile per head pair:
                        # independent PE filler work across the seam
                        ib_o = ib_order[ib_i - 1] if ib_i > 0 else None
                        if ib_o is not None:
                            it = 4 * ib_o + pr
                            itsl = slice(it * 128, (it + 1) * 128)
                            for ob in range(2):
                                osl = slice(ob * SB, (ob + 1) * SB)
                                ps = accp.tile([128, SB], F32, tag="acc")
                                for gg in range(NEG):
                                    nc.tensor.matmul(
                                        out=ps,
                                        lhsT=(ctxT[:, gg, itsl]),
                                        rhs=(woT_sb[:, gg, osl]),
                                        start=(gg == 0),
                                        stop=(gg == NEG - 1),
                                    )
                                ot = p3.tile([128, SB], F32, tag="ot")
                                nc.vector.tensor_copy(ot, ps)
                                nc.sync.dma_start(out=out_d[itsl, osl], in_=ot)


                # tail: output projection for the last-processed i-block
                for it in range(4 * ib_order[-1], 4 * ib_order[-1] + 4):
                    itsl = slice(it * 128, (it + 1) * 128)
                    for ob in range(2):
                        osl = slice(ob * SB, (ob + 1) * SB)
                        ps = accp.tile([128, SB], F32, tag="acc")
                        for gg in range(NEG):
                            nc.tensor.matmul(
                                out=ps,
                                lhsT=(ctxT[:, gg, itsl]),
                                rhs=(woT_sb[:, gg, osl]),
                                start=(gg == 0),
                                stop=(gg == NEG - 1),
                            )
                        ot = p3.tile([128, SB], F32, tag="ot")
                        nc.vector.tensor_copy(ot, ps)
                        nc.sync.dma_start(out=out_d[itsl, osl], in_=ot)

    nc.finalize()
    return nc


_NC = None


def _get_nc():
    global _NC
    if _NC is None:
        _NC = _build()
    return _NC


def kernel(x, wq, wk, wv, wo, wo_b):
    global LAST_RESULT
    x = np.ascontiguousarray(np.asarray(x, dtype=np.float32))
    wq = np.asarray(wq, dtype=np.float32)
    wk = np.asarray(wk, dtype=np.float32)
    wv = np.asarray(wv, dtype=np.float32)
    wo = np.asarray(wo, dtype=np.float32)
    wo_b = np.asarray(wo_b, dtype=np.float32)

    pp, ff = np.ogrid[0:128, 0:128]
    masks = (pp <= ff).astype(np.float32)

    in_maps = []
    for c in range(NCORES):
        b, hh = c // 2, c % 2
        es = slice(hh * EH, (hh + 1) * EH)
        in_maps.append(
            {
                "xt": np.ascontiguousarray(x[b].T.astype(MMNP)),
                "wqt": np.ascontiguousarray(wq[es, :].T.astype(MMNP)),
                "wkt": np.ascontiguousarray(wk[es, :].T.astype(MMNP)),
                "wvt": np.ascontiguousarray(wv[es, :].T.astype(MMNP)),
                "wot": np.ascontiguousarray(
                    wo[:, es].T.astype(MMNP)
                    .reshape(4, 2, 64, D)[:, ::-1]
                    .reshape(EH, D)
                ),
                "masks": masks.astype(MMNP),
            }
        )

    nc = _get_nc()
    res = run_bass_kernel_spmd(nc, in_maps, list(range(NCORES)), trace=TRACE)
    LAST_RESULT = res

    out = np.empty((B, S, D), np.float32)
    for b in range(B):
        out[b] = res.results[2 * b]["out"] + res.results[2 * b + 1]["out"]
    out += wo_b[None, None, :]
    return out


# revision 32
# speedup vs baseline: 1.1750x; 1.0007x over previous
"""Multi-head causal attention on 8 Trainium2 NeuronCores.

Sharding: core c -> (batch b = c//2, head-half hh = c%2).  Each core computes
q/k/v projections for its 8 heads (column-sharded wq/wk/wv), causal attention,
and a full-width partial output projection (row-sharded wo).  Host sums the
two partials per batch and adds the bias.

Device-side layout trick: scores are computed transposed (scoresT[j, i]) so
that the softmax-weighted sum over keys (ctx) is a plain matmul with v as the
stationary operand.  Ones-columns baked alongside v produce the softmax
denominator replicated across 64 partitions in the same PSUM tile as ctx.
"""

import numpy as np

import concourse.bass as bass
import concourse.mybir as mybir
import concourse.tile as tile
from concourse import bacc
from concourse.bass_utils import run_bass_kernel_spmd

# Problem shape (hardcoded; kernel.py must be self-contained).
B, S, D, H = 4, 2048, 1024, 16
HD = D // H           # 64 head dim
NCORES = 8
EH = D // 2           # 512: per-core e-width (8 heads)
NHL = H // 2          # 8 local heads per core
SB = 512              # s-block (free dim of most matmuls)
NSB = S // SB         # 4
NST = S // 128        # 16 s-tiles / j-tiles
NEG = EH // 128       # 4 e-groups of 128 partitions
NKG = D // 128        # 8 d-groups (contraction tiles)
VROW = 4 * 192        # v_ext row: 4x [v_even(64) | ones(64) | v_odd(64)] = 768

F32 = mybir.dt.float32
F32R = mybir.dt.float32r
BF16 = mybir.dt.bfloat16
MMDT = F32R          # dtype for matmul inputs (BF16 or F32R)
import ml_dtypes
MMNP = ml_dtypes.bfloat16 if MMDT == BF16 else np.float32

TRACE = False
LAST_RESULT = None


def _build():
    nc = bacc.Bacc()

    xT_d = nc.dram_tensor("xt", [D, S], MMDT, kind="ExternalInput")
    wqT_d = nc.dram_tensor("wqt", [D, EH], MMDT, kind="ExternalInput")
    wkT_d = nc.dram_tensor("wkt", [D, EH], MMDT, kind="ExternalInput")
    wvT_d = nc.dram_tensor("wvt", [D, EH], MMDT, kind="ExternalInput")
    woT_d = nc.dram_tensor("wot", [EH, D], MMDT, kind="ExternalInput")
    masks_d = nc.dram_tensor("masks", [128, 128], MMDT, kind="ExternalInput")
    out_d = nc.dram_tensor("out", [S, D], F32, kind="ExternalOutput")
    scr_d = nc.dram_tensor("dscr", [4, NSB, 2, SB], F32)

    with tile.TileContext(nc) as tc:
        with (
            tc.tile_pool(name="persist", bufs=1) as persist,
            tc.tile_pool(name="accp", bufs=4, space="PSUM") as accp,
        ):
            qT = persist.tile([128, NEG, S], MMDT)      # [e-part, e-group, s]
            kT = persist.tile([128, NEG, S], MMDT)
            v_ext = persist.tile([128, NST, VROW], MMDT)  # [s-part, s-tile, row]

            # shared ones block between each (even, odd) head pair
            for st in range(NST):
                for p in range(4):
                    ones_ap = v_ext[:, st, p * 192 + 64 : p * 192 + 128]
                    if MMDT == F32R:
                        ones_ap = ones_ap.bitcast(F32)
                    nc.vector.memset(ones_ap, 1.0)

            # ---------------- Phase 1: projections ----------------
            with (
                tc.tile_pool(name="p1w", bufs=1) as p1w,
                tc.tile_pool(name="p1x", bufs=2) as p1x,
            ):
                w_q = p1w.tile([128, NKG, EH], MMDT)
                w_k = p1w.tile([128, NKG, EH], MMDT)
                w_v = p1w.tile([128, NKG, EH], MMDT)
                for kg in range(NKG):
                    sl = slice(kg * 128, (kg + 1) * 128)
                    nc.gpsimd.dma_start(out=w_q[:, kg, :], in_=wqT_d[sl, :])
                for kg in range(NKG):
                    sl = slice(kg * 128, (kg + 1) * 128)
                    nc.gpsimd.dma_start(out=w_k[:, kg, :], in_=wkT_d[sl, :])
                    nc.gpsimd.dma_start(out=w_v[:, kg, :], in_=wvT_d[sl, :])

                for sb in range(NSB):
                    ssl = slice(sb * SB, (sb + 1) * SB)
                    xts = p1x.tile([128, NKG, SB], MMDT, tag="xts")
                    for kg in range(NKG):
                        nc.sync.dma_start(
                            out=xts[:, kg, :],
                            in_=xT_d[kg * 128 : (kg + 1) * 128, ssl],
                        )
                    # qT / kT blocks: out [e-part(128 of group mt), s(512)]
                    for w_sb, dst in ((w_q, qT), (w_k, kT)):
                        for mt in range(NEG):
                            ps = accp.tile([128, SB], F32, tag="acc")
                            msl = slice(mt * 128, (mt + 1) * 128)
                            for kg in range(NKG):
                                nc.tensor.matmul(
                                    out=ps,
                                    lhsT=(w_sb[:, kg, msl]),
                                    rhs=(xts[:, kg, :]),
                                    start=(kg == 0),
                                    stop=(kg == NKG - 1),
                                )
                            nc.vector.tensor_copy(dst[:, mt, ssl], ps)
                    # v blocks: out [s-part(128 of tile st), e(512)]
                    for st4 in range(SB // 128):
                        st = sb * (SB // 128) + st4
                        ps = accp.tile([128, EH], F32, tag="acc")
                        xsl = slice(st4 * 128, (st4 + 1) * 128)
                        for kg in range(NKG):
                            nc.tensor.matmul(
                                out=ps,
                                lhsT=(xts[:, kg, xsl]),
                                rhs=(w_v[:, kg, :]),
                                start=(kg == 0),
                                stop=(kg == NKG - 1),
                            )
                        # psum cols: head h at [h*64, h*64+64); dest pair p:
                        # even head -> p*192, odd head -> p*192+128
                        psr = ps[:].rearrange("p (a c) -> p a c", c=128)
                        vst = v_ext[:, st, :].rearrange("p (a w) -> p a w", w=192)
                        nc.vector.tensor_copy(vst[:, :, 128:192], psr[:, :, 0:64])
                        nc.vector.tensor_copy(vst[:, :, 0:64], psr[:, :, 64:128])

            # ---------------- Phase 2 + 3: attention and output proj ----------------
            # i-blocks outer, head pairs inner; once an i-block has all 8
            # heads' context, its output-projection tiles run immediately so
            # phase-3 matmuls and output DMAs overlap the attention phase.
            with (
                tc.tile_pool(name="p2c", bufs=1) as p2c,
                tc.tile_pool(name="ctxp", bufs=1) as ctxp,
                tc.tile_pool(name="expp", bufs=4) as expp,
                tc.tile_pool(name="sp", bufs=2, space="PSUM") as sp,
                tc.tile_pool(name="smallp", bufs=2) as smallp,
                tc.tile_pool(name="p3", bufs=2) as p3,
            ):
                masks_sb = p2c.tile([128, 128], MMDT)
                nc.gpsimd.dma_start(out=masks_sb, in_=masks_d[:, :])
                woT_sb = p2c.tile([128, NEG, D], MMDT)
                for gg in range(NEG):
                    nc.gpsimd.dma_start(
                        out=woT_sb[:, gg, :],
                        in_=woT_d[gg * 128 : (gg + 1) * 128, :],
                    )

                ctxT = ctxp.tile([128, NEG, S], MMDT)
                ib_order = [0, 1, 2, 3]
                for ib_i, ib in enumerate(ib_order):
                    isl = slice(ib * SB, (ib + 1) * SB)
                    njt = 4 * (ib + 1)
                    for pr in range(4):
                        ps_c0 = accp.tile([128, SB], F32, tag="acc")
                        ps_c1 = accp.tile([128, SB], F32, tag="acc")

                        def scores(jt):
                            r = jt - 4 * ib
                            f0 = 128 * r if r > 0 else 0
                            jsl = slice(jt * 128, (jt + 1) * 128)
                            qsl = slice(ib * SB + f0, (ib + 1) * SB)
                            ps_s = sp.tile([128, 2 * SB], F32, tag="s")
                            nc.tensor.matmul(
                                out=ps_s[:, f0:SB],
                                lhsT=kT[0:64, pr, jsl],
                                rhs=qT[0:64, pr, qsl],
                                start=True,
                                stop=True,
                            )
                            nc.tensor.matmul(
                                out=ps_s[:, SB + f0 : 2 * SB],
                                lhsT=kT[64:128, pr, jsl],
                                rhs=qT[64:128, pr, qsl],
                                start=True,
                                stop=True,
                            )
                            return ps_s

                        def softmax_ctx(jt, ps_s):
                            r = jt - 4 * ib
                            f0 = 128 * r if r > 0 else 0
                            expT = expp.tile([128, 2 * SB], MMDT, tag="exp")
                            ps_v = ps_s[:].rearrange("p (t c) -> p t c", t=2)
                            ex_v = expT[:].rearrange("p (t c) -> p t c", t=2)
                            nc.scalar.activation(
                                out=ex_v[:, :, f0:SB],
                                in_=ps_v[:, :, f0:SB],
                                func=mybir.ActivationFunctionType.Exp,
                                scale=1.0 / np.sqrt(HD),
                            )
                            if r >= 0:
                                nc.vector.tensor_mul(
                                    ex_v[:, :, f0 : f0 + 128],
                                    ex_v[:, :, f0 : f0 + 128],
                                    masks_sb[:].unsqueeze(1).broadcast_to(
                                        (128, 2, 128)
                                    ),
                                )
                            for t, ps_c in ((0, ps_c0), (1, ps_c1)):
                                coff = pr * 192 + (64 if t == 0 else 0)
                                nc.tensor.matmul(
                                    out=ps_c[:, f0:SB],
                                    lhsT=v_ext[:, jt, coff : coff + 128],
                                    rhs=expT[:, t * SB + f0 : (t + 1) * SB],
                                    start=(jt == 0),
                                    stop=(jt == njt - 1),
                                )

                        prev = None
                        for jt in range(njt):
                            ps_prev = prev
                            prev = (jt, scores(jt))
                            if ps_prev is not None:
                                softmax_ctx(*ps_prev)
                        softmax_ctx(*prev)

                        # even head (ps_c0): denom rows 0:64, ctx rows 64:128
                        den0 = smallp.tile([128, SB], F32, tag="den0")
                        nc.vector.tensor_copy(den0[0:64, :], ps_c0[0:64, :])
                        rdt0 = smallp.tile([128, SB], F32, tag="rdt0")
                        nc.vector.reciprocal_approx_fast(
                            rdt0[0:64, :], den0[0:64, :]
                        )
                        nc.sync.dma_start(
                            out=scr_d[pr, ib, 0, :], in_=rdt0[0:1, :]
                        )
                        se = scr_d[pr, ib, 0, :]
                        bce = smallp.tile([128, SB], F32, tag="bce")
                        nc.sync.dma_start(
                            out=bce[64:128, :],
                            in_=bass.AP(
                                tensor=se.tensor, offset=se.offset,
                                ap=[[0, 64], [1, SB]],
                            ),
                        )
                        nc.vector.tensor_mul(
                            ctxT[64:128, pr, isl], ps_c0[64:128, :], bce[64:128, :]
                        )
                        # odd head (ps_c1): ctx rows 0:64, denom rows 64:128
                        den1 = smallp.tile([128, SB], F32, tag="den1")
                        nc.vector.tensor_copy(den1[64:65, :], ps_c1[64:65, :])
                        nc.sync.dma_start(
                            out=scr_d[pr, ib, 1, :], in_=den1[64:65, :]
                        )
                        so = scr_d[pr, ib, 1, :]
                        braw = smallp.tile([128, SB], F32, tag="braw")
                        nc.sync.dma_start(
                            out=braw[0:64, :],
                            in_=bass.AP(
                                tensor=so.tensor, offset=so.offset,
                                ap=[[0, 64], [1, SB]],
                            ),
                        )
                        rdt1 = smallp.tile([128, SB], F32, tag="rdt1")
                        nc.vector.reciprocal_approx_fast(
                            rdt1[0:64, :], braw[0:64, :]
                        )
                        nc.vector.tensor_mul(
                            ctxT[0:64, pr, isl], ps_c1[0:64, :], rdt1[0:64, :]
                        )

                    # output projection, deferred one i-block so its
                    # dependencies (normalize chain) are already settled
                    ib_o = ib_order[ib_i - 1] if ib_i > 0 else None
                    for it in ([] if ib_o is None else range(4 * ib_o, 4 * ib_o + 4)):
                        itsl = slice(it * 128, (it + 1) * 128)
                        for ob in range(2):
                            osl = slice(ob * SB, (ob + 1) * SB)
                            ps = accp.tile([128, SB], F32, tag="acc")
                            for gg in range(NEG):
                                nc.tensor.matmul(
                                    out=ps,
                                    lhsT=(ctxT[:, gg, itsl]),
                                    rhs=(woT_sb[:, gg, osl]),
                                    start=(gg == 0),
                                    stop=(gg == NEG - 1),
                                )
                            ot = p3.tile([128, SB], F32, tag="ot")
                            nc.vector.tensor_copy(ot, ps)
                            nc.sync.dma_start(out=out_d[itsl, osl], in_=ot)


                # tail: output projection for the last-processed i-block
                for it in range(4 * ib_order[-1], 4 * ib_order[-1] + 4):
                    itsl = slice(it * 128, (it + 1) * 128)
                    for ob in range(2):
                        osl = slice(ob * SB, (ob + 1) * SB)
                        ps = accp.tile([128, SB], F32, tag="acc")
                        for gg in range(NEG):
                            nc.tensor.matmul(
                                out=ps,
                                lhsT=(ctxT[:, gg, itsl]),
                                rhs=(woT_sb[:, gg, osl]),
                                start=(gg == 0),
                                stop=(gg == NEG - 1),
                            )
                        ot = p3.tile([128, SB], F32, tag="ot")
                        nc.vector.tensor_copy(ot, ps)
                        nc.sync.dma_start(out=out_d[itsl, osl], in_=ot)

    nc.finalize()
    return nc


_NC = None


def _get_nc():
    global _NC
    if _NC is None:
        _NC = _build()
    return _NC


def kernel(x, wq, wk, wv, wo, wo_b):
    global LAST_RESULT
    x = np.ascontiguousarray(np.asarray(x, dtype=np.float32))
    wq = np.asarray(wq, dtype=np.float32)
    wk = np.asarray(wk, dtype=np.float32)
    wv = np.asarray(wv, dtype=np.float32)
    wo = np.asarray(wo, dtype=np.float32)
    wo_b = np.asarray(wo_b, dtype=np.float32)

    pp, ff = np.ogrid[0:128, 0:128]
    masks = (pp <= ff).astype(np.float32)

    in_maps = []
    for c in range(NCORES):
        b, hh = c // 2, c % 2
        es = slice(hh * EH, (hh + 1) * EH)
        in_maps.append(
            {
                "xt": np.ascontiguousarray(x[b].T.astype(MMNP)),
                "wqt": np.ascontiguousarray(wq[es, :].T.astype(MMNP)),
                "wkt": np.ascontiguousarray(wk[es, :].T.astype(MMNP)),
                "wvt": np.ascontiguousarray(wv[es, :].T.astype(MMNP)),
                "wot": np.ascontiguousarray(
                    wo[:, es].T.astype(MMNP)
                    .reshape(4, 2, 64, D)[:, ::-1]
                    .reshape(EH, D)
                ),
                "masks": masks.astype(MMNP),
            }
        )

    nc = _get_nc()
    res = run_bass_kernel_spmd(nc, in_maps, list(range(NCORES)), trace=TRACE)
    LAST_RESULT = res

    out = np.empty((B, S, D), np.float32)
    for b in range(B):
        out[b] = res.results[2 * b]["out"] + res.results[2 * b + 1]["out"]
    out += wo_b[None, None, :]
    return out
